# revision 12
# baseline (speedup 1.0000x reference)
"""2D Haar DWT (single level) on Trainium2, 8 NeuronCores, pure data parallel.

Math: per-2x2-block butterflies (ll,lh,hl,hh) = 0.5*(x00 +/- x01 +/- x10
+/- x11).  bf16 crosses HBM both ways (host casts; *0.5 folded into the
PSUM evacuation): 4 MiB in + 4 MiB out per core.

The WHOLE transform is one matmul per 1024-column chunk: the host puts
(row parity rp, column parity t) on the PARTITION axis —
p = rp*64 + t*32 + mm, image row = 2*m+rp with m = 32*g8+mm,
col = 2k+t — so each output element is
a +/-1 combination of 4 partitions with equal (mm): a single stationary
matrix B4[128,128] with B4[rp*64+t*32+mm, (2c+h)*32+mm] = s_c[rp]*s_h[t]
(s_0=[1,1], s_1=[1,-1]) computes ALL FOUR subbands at once.

v4 schedule, from trace findings:
 (a) SDMA cost is per-PACKET (128 descriptors + 16 sem packets per
     [128,N] DMA, ~250ns/packet feed, 8 KiB data max per packet): so
     FEW, LARGE transfers win.  B4 is EMBEDDED in the input tensor
     (first 128 cols) instead of its own 128x256B-packet DMA, in-DMAs
     are [1,2,2,2,1]-g8 (4-8.5 KiB runs), out-DMAs are 5 large slices.
 (b) each [128,1024] PSUM chunk is ONE 1024-col bf16 matmul (moving max
     for bf16), halving PE time vs 2x512, and is evacuated as two
     512-col halves on DVE and ACT IN PARALLEL (~0.69us each).
 (c) out-DMAs ride BOTH HWDGE rings: the two early slices on the ACT
     ring (empty early), the three late slices on the SP ring, whose
     FIFO drains them right after the ins - in and out streams overlap
     across the whole window instead of serializing.

Out DRAM is written in SBUF-native order; the host un-permutes.
"""

import numpy as np
import ml_dtypes

import concourse.mybir as mybir
from concourse import bacc, tile
from concourse.bass_utils import run_bass_kernel_spmd

N_CORES = 8
BATCH = 64
B_PER = BATCH // N_CORES  # 8 images per core
H = W = 512

BF16 = ml_dtypes.bfloat16
NCOL = 16384  # data columns per core
XOFF = 128  # B4 occupies the first 128 columns of the input tensor

# in-DMA slices as g8-block ranges (1 g8-block = 2048 cols, 4 KiB runs);
# the first slice also carries the embedded B4 block
IN_SLICES = [(0, 1), (1, 3), (3, 5), (5, 7), (7, 8)]

# out-DMA slices: (start col, end col, ring).  All outs ride the SP ring:
# the SDMA feed serves one HWDGE ring at a time anyway, the SP FIFO drains
# outs right behind the ins with no ring-handover bubble, and the ACT
# sequencer keeps only evacuation work (no 0.6us trigger stalls)
OUT_SLICES = [
    (0, 2048, "sync"),
    (2048, 6144, "sync"),
    (6144, 10240, "sync"),
    (10240, 14336, "sync"),
    (14336, 16384, "sync"),
]

_nc_cache = None


def build_bass():
    bf16 = mybir.dt.bfloat16
    f32 = mybir.dt.float32
    nc = bacc.Bacc(
        "TRN2", target_bir_lowering=False, debug=False, num_devices=N_CORES
    )
    # col 0..127: B4; col 128+: [g8][k][j] data (p = rp*64+t*32+mm)
    inp = nc.dram_tensor(
        "inputs", [128, XOFF + NCOL], bf16, kind="ExternalInput"
    ).ap()
    # [p' = (2c+h)*32+mm][g8][k][j]
    out = nc.dram_tensor("out", [128, NCOL], bf16, kind="ExternalOutput").ap()

    with tile.TileContext(nc) as tc:
        pool_cm = tc.tile_pool(name="p", bufs=1)
        pool = pool_cm.__enter__()
        ps_cm = tc.psum_pool(name="ps", bufs=4)
        psp = ps_cm.__enter__()

        lp_cm = nc.allow_low_precision(reason="bf16 DWT: rel-err budget 2e-2")
        lp_cm.__enter__()

        X = pool.tile([128, XOFF + NCOL], bf16, tag="X", bufs=1)
        Yb = pool.tile([128, NCOL], bf16, tag="Yb", bufs=1)
        B = X[:, 0:XOFF]

        # all in-DMAs up-front on the SP ring: disjoint ranges of one tile,
        # no anti-deps, the ring streams them back-to-back at line rate
        for g0, g1 in IN_SLICES:
            a = XOFF + g0 * 2048 if g0 else 0
            b = XOFF + g1 * 2048
            nc.sync.dma_start(out=X[:, a:b], in_=inp[:, a:b])

        out_iter = iter(OUT_SLICES)
        next_out = next(out_iter)

        for i in range(16):
            c0 = 1024 * i
            ps = psp.tile([128, 1024], f32, tag="ps")
            nc.tensor.matmul(ps[:, 0:512], B, X[:, XOFF + c0 : XOFF + c0 + 512])
            nc.tensor.matmul(
                ps[:, 512:1024], B, X[:, XOFF + c0 + 512 : XOFF + c0 + 1024]
            )
            # whole-chunk evacuation, DVE/ACT alternating (both stay ~50% busy
            # and neither serializes behind out-DMA triggers)
            dst = Yb[:, c0 : c0 + 1024]
            if i % 2 == 0:
                nc.vector.tensor_scalar_mul(dst, ps[:], 0.5)
            else:
                nc.scalar.mul(dst, ps[:], 0.5)

            while next_out is not None and next_out[1] <= c0 + 1024:
                a, b, ring = next_out
                eng = nc.scalar if ring == "act" else nc.sync
                eng.dma_start(out=out[:, a:b], in_=Yb[:, a:b])
                next_out = next(out_iter, None)

        lp_cm.__exit__(None, None, None)
        ps_cm.__exit__(None, None, None)
        pool_cm.__exit__(None, None, None)

    nc.compile()
    return nc


def _bmat_block():
    b = np.zeros((128, 128), dtype=np.float32)
    mm = np.arange(32)
    sgn = [np.array([1.0, 1.0]), np.array([1.0, -1.0])]
    for rp in range(2):
        for t in range(2):
            for c in range(2):
                for h in range(2):
                    b[rp * 64 + t * 32 + mm, (2 * c + h) * 32 + mm] = (
                        sgn[c][rp] * sgn[h][t]
                    )
    return b.astype(BF16)


def prep_inputs(x):
    """x: (64, 512, 512) f32 -> per-core [128, 128+16384] bf16 (B4-prefixed)."""
    # [B][g8][mm][rp][k][t]: row = 2*(32*g8+mm)+rp, col = 2k+t
    arr = np.asarray(x, dtype=np.float32).reshape(BATCH, 8, 32, 2, 256, 2)
    arr = arr.astype(BF16)
    bm = _bmat_block()
    shards = []
    for c in range(N_CORES):
        blk = arr[c * B_PER : (c + 1) * B_PER]  # [j][g8][mm][rp][k][t]
        blk = blk.transpose(3, 5, 2, 1, 4, 0)  # [rp][t][mm][g8][k][j]
        data = np.ascontiguousarray(blk).reshape(128, NCOL)
        shards.append(np.ascontiguousarray(np.concatenate([bm, data], axis=1)))
    return shards


def assemble_output(outs):
    """outs: per-core [128, 16384] bf16 -> (64, 512, 512, 1) f32 (scaled)."""
    res = np.empty((BATCH, H, W), dtype=np.float32)
    for core, o in enumerate(outs):
        # [c][h][mm][g8][k][j] -> [j][c][g8][mm][h][k]
        blk = o.reshape(2, 2, 32, 8, 256, 8).transpose(5, 0, 3, 2, 1, 4)
        res[core * B_PER : (core + 1) * B_PER] = blk.reshape(B_PER, H, W)
    return res.reshape(BATCH, H, W, 1)


def kernel(**inputs):
    global _nc_cache
    x = np.asarray(inputs["inputs"], dtype=np.float32).reshape(BATCH, H, W)
    shards = prep_inputs(x)
    if _nc_cache is None:
        _nc_cache = build_bass()
    nc = _nc_cache
    in_maps = [{"inputs": shards[i]} for i in range(N_CORES)]
    res = run_bass_kernel_spmd(nc, in_maps, core_ids=list(range(N_CORES))).results
    return assemble_output([res[i]["out"] for i in range(N_CORES)])


# revision 13
# speedup vs baseline: 1.1004x; 1.1004x over previous
"""2D Haar DWT (single level) on Trainium2, 8 NeuronCores, pure data parallel.

Math: per-2x2-block butterflies (ll,lh,hl,hh) = 0.5*(x00 +/- x01 +/- x10
+/- x11).  bf16 crosses HBM both ways (host casts; *0.5 folded into the
PSUM evacuation): 4 MiB in + 4 MiB out per core.

The WHOLE transform is one matmul per 1024-column chunk: the host puts
(row parity rp, column parity t) on the PARTITION axis —
p = rp*64 + t*32 + mm, image row = 2*m+rp with m = 32*g8+mm,
col = 2k+t — so each output element is
a +/-1 combination of 4 partitions with equal (mm): a single stationary
matrix B4[128,128] with B4[rp*64+t*32+mm, (2c+h)*32+mm] = s_c[rp]*s_h[t]
(s_0=[1,1], s_1=[1,-1]) computes ALL FOUR subbands at once.

v4 schedule, from trace findings:
 (a) SDMA cost is per-PACKET (128 descriptors + 16 sem packets per
     [128,N] DMA, ~250ns/packet feed, 8 KiB data max per packet): so
     FEW, LARGE transfers win.  B4 is EMBEDDED in the input tensor
     (first 128 cols) instead of its own 128x256B-packet DMA, in-DMAs
     are [1,2,2,2,1]-g8 (4-8.5 KiB runs), out-DMAs are 5 large slices.
 (b) each [128,1024] PSUM chunk is ONE 1024-col bf16 matmul (moving max
     for bf16), halving PE time vs 2x512, and is evacuated as two
     512-col halves on DVE and ACT IN PARALLEL (~0.69us each).
 (c) out-DMAs ride BOTH HWDGE rings: the two early slices on the ACT
     ring (empty early), the three late slices on the SP ring, whose
     FIFO drains them right after the ins - in and out streams overlap
     across the whole window instead of serializing.

Out DRAM is written in SBUF-native order; the host un-permutes.
"""

import numpy as np
import ml_dtypes

import concourse.mybir as mybir
from concourse import bacc, tile
from concourse.bass_utils import run_bass_kernel_spmd

N_CORES = 8
BATCH = 64
B_PER = BATCH // N_CORES  # 8 images per core
H = W = 512

BF16 = ml_dtypes.bfloat16
NCOL = 16384  # data columns per core
XOFF = 128  # B4 occupies the first 128 columns of the input tensor

# in-DMA slices as g8-block ranges (1 g8-block = 2048 cols, 4 KiB runs);
# the first slice also carries the embedded B4 block
IN_SLICES = [(0, 1), (1, 3), (3, 5), (5, 7), (7, 8)]

# out-DMA slices: (start col, end col, ring).  All outs ride the SP ring:
# the SDMA feed serves one HWDGE ring at a time anyway, the SP FIFO drains
# outs right behind the ins with no ring-handover bubble, and the ACT
# sequencer keeps only evacuation work (no 0.6us trigger stalls)
OUT_SLICES = [
    (0, 2048, "sync"),
    (2048, 6144, "sync"),
    (6144, 14336, "sync"),
    (14336, 16384, "sync"),
]

_nc_cache = None


def build_bass():
    bf16 = mybir.dt.bfloat16
    f32 = mybir.dt.float32
    nc = bacc.Bacc(
        "TRN2", target_bir_lowering=False, debug=False, num_devices=N_CORES
    )
    # col 0..127: B4; col 128+: [g8][k][j] data (p = rp*64+t*32+mm)
    inp = nc.dram_tensor(
        "inputs", [128, XOFF + NCOL], bf16, kind="ExternalInput"
    ).ap()
    # [p' = (2c+h)*32+mm][g8][k][j]
    out = nc.dram_tensor("out", [128, NCOL], bf16, kind="ExternalOutput").ap()

    with tile.TileContext(nc) as tc:
        pool_cm = tc.tile_pool(name="p", bufs=1)
        pool = pool_cm.__enter__()
        ps_cm = tc.psum_pool(name="ps", bufs=4)
        psp = ps_cm.__enter__()

        lp_cm = nc.allow_low_precision(reason="bf16 DWT: rel-err budget 2e-2")
        lp_cm.__enter__()

        X = pool.tile([128, XOFF + NCOL], bf16, tag="X", bufs=1)
        Yb = pool.tile([128, NCOL], bf16, tag="Yb", bufs=1)
        B = X[:, 0:XOFF]

        # all in-DMAs up-front on the SP ring: disjoint ranges of one tile,
        # no anti-deps, the ring streams them back-to-back at line rate
        for g0, g1 in IN_SLICES:
            a = XOFF + g0 * 2048 if g0 else 0
            b = XOFF + g1 * 2048
            nc.sync.dma_start(out=X[:, a:b], in_=inp[:, a:b])

        out_iter = iter(OUT_SLICES)
        next_out = next(out_iter)

        for i in range(16):
            c0 = 1024 * i
            ps = psp.tile([128, 1024], f32, tag="ps")
            nc.tensor.matmul(ps[:, 0:512], B, X[:, XOFF + c0 : XOFF + c0 + 512])
            nc.tensor.matmul(
                ps[:, 512:1024], B, X[:, XOFF + c0 + 512 : XOFF + c0 + 1024]
            )
            # whole-chunk evacuation, DVE/ACT alternating (both stay ~50% busy
            # and neither serializes behind out-DMA triggers)
            dst = Yb[:, c0 : c0 + 1024]
            if i % 2 == 0:
                nc.vector.tensor_scalar_mul(dst, ps[:], 0.5)
            else:
                nc.scalar.mul(dst, ps[:], 0.5)

            while next_out is not None and next_out[1] <= c0 + 1024:
                a, b, ring = next_out
                eng = nc.scalar if ring == "act" else nc.sync
                eng.dma_start(out=out[:, a:b], in_=Yb[:, a:b])
                next_out = next(out_iter, None)

        lp_cm.__exit__(None, None, None)
        ps_cm.__exit__(None, None, None)
        pool_cm.__exit__(None, None, None)

    nc.compile()
    return nc


def _bmat_block():
    b = np.zeros((128, 128), dtype=np.float32)
    mm = np.arange(32)
    sgn = [np.array([1.0, 1.0]), np.array([1.0, -1.0])]
    for rp in range(2):
        for t in range(2):
            for c in range(2):
                for h in range(2):
                    b[rp * 64 + t * 32 + mm, (2 * c + h) * 32 + mm] = (
                        sgn[c][rp] * sgn[h][t]
                    )
    return b.astype(BF16)


def prep_inputs(x):
    """x: (64, 512, 512) f32 -> per-core [128, 128+16384] bf16 (B4-prefixed)."""
    # [B][g8][mm][rp][k][t]: row = 2*(32*g8+mm)+rp, col = 2k+t
    arr = np.asarray(x, dtype=np.float32).reshape(BATCH, 8, 32, 2, 256, 2)
    arr = arr.astype(BF16)
    bm = _bmat_block()
    shards = []
    for c in range(N_CORES):
        blk = arr[c * B_PER : (c + 1) * B_PER]  # [j][g8][mm][rp][k][t]
        blk = blk.transpose(3, 5, 2, 1, 4, 0)  # [rp][t][mm][g8][k][j]
        data = np.ascontiguousarray(blk).reshape(128, NCOL)
        shards.append(np.ascontiguousarray(np.concatenate([bm, data], axis=1)))
    return shards


def assemble_output(outs):
    """outs: per-core [128, 16384] bf16 -> (64, 512, 512, 1) f32 (scaled)."""
    res = np.empty((BATCH, H, W), dtype=np.float32)
    for core, o in enumerate(outs):
        # [c][h][mm][g8][k][j] -> [j][c][g8][mm][h][k]
        blk = o.reshape(2, 2, 32, 8, 256, 8).transpose(5, 0, 3, 2, 1, 4)
        res[core * B_PER : (core + 1) * B_PER] = blk.reshape(B_PER, H, W)
    return res.reshape(BATCH, H, W, 1)


def kernel(**inputs):
    global _nc_cache
    x = np.asarray(inputs["inputs"], dtype=np.float32).reshape(BATCH, H, W)
    shards = prep_inputs(x)
    if _nc_cache is None:
        _nc_cache = build_bass()
    nc = _nc_cache
    in_maps = [{"inputs": shards[i]} for i in range(N_CORES)]
    res = run_bass_kernel_spmd(nc, in_maps, core_ids=list(range(N_CORES))).results
    return assemble_output([res[i]["out"] for i in range(N_CORES)])
